# revision 1
# baseline (speedup 1.0000x reference)
"""Multi-head attention kernel for Trainium2, 8 NeuronCores.

Problem: B=4, S=2048, D=1024, H=16 heads, d_k=64 (fp32).
    out = softmax((Q Wq + bq)(K Wk + bk)^T / 8) (V Wv + bv) Wo + bo

Sharding: core c handles batch b = c//2 and head-group g = c%2
(8 heads, a 512-wide slice of the model dim). W_q/W_k/W_v are split
column-wise, W_o row-wise; each core computes a full [2048, 1024]
partial output and the host sums core pairs and adds bo.

Per-core dataflow (everything fp32):
  1. PE-transpose X (the relevant input) into X^T slices (in-dim on
     partitions) per 512-column sequence chunk.
  2. Projections: q^T, k^T produced transposed (head-dim on partitions,
     bias via per-partition tensor_scalar add; 1/sqrt(d_k) folded into
     Wq/bq on the host); v produced in natural orientation with bias
     via a ones-row matmul, stored ones-augmented ([v | 1] per 128-row
     chunk) so the attn@V matmul also produces softmax denominators.
  3. Attention per head: S^T tiles = k^T.T @ q^T (keys on partitions),
     exp on ACT straight out of PSUM (no max subtraction needed:
     scores ~ N(0,1), fp32 exp overflows only beyond 88), then
     O^T = [v|1].T @ exp(S^T) accumulated over key chunks; row 64 of
     the [65, 512] result is the softmax denominator.
  4. Normalize O^T by reciprocal denominators (broadcast across
     partitions via a DRAM round-trip DMA), then the output projection
     contracts the packed O^T tiles against Wo rows.
"""

import sys

sys.path.insert(0, '/opt/trn_rl_repo')

import numpy as np

B = 4
S = 2048
D = 1024
H = 16
DK = 64
HPC = 8          # heads per core
DH = 512         # model-dim slice per core (HPC * DK)
N_CORES = 8
SC = 512         # sequence chunk for projections
NSC = S // SC    # 4
NKB = S // 128   # 16 key blocks
NQC = S // 512   # 4 query chunks

_CACHE = {}


def _build():
    import concourse.bass as bass
    import concourse.tile as tile
    from concourse import mybir
    from concourse.masks import make_identity
    import bass_rust

    # ---- workarounds for this walrus build: max ONE sync wait/instr ----
    def _patched_drain_and_barrier(self, tick_clock, wait_clock):
        drain_inst = self.nc.sync.drain()
        wait_clock.add_sem_waits(
            drain_inst.ins, tile.ScopedClock({None: tick_clock.global_clock}))
        mi = drain_inst.ins
        si = mi.sync_info
        waits = list(si.on_wait or []) if si is not None else []
        if len(waits) > 1:
            si.on_wait = waits[:1]
            for w in waits[1:]:
                d2 = self.nc.sync.drain()
                si2 = d2.ins.sync_info
                if si2 is None:
                    d2.ins.sync_info = bass_rust.SyncInfo(on_wait=[w], on_update=[])
                else:
                    si2.on_wait = [w]
        self.nc.all_engine_barrier()
        popped = self.nc._tile_sem_poison_stack.pop()
        assert popped is self._sem_poison
        self.nc.clear_and_free_semaphores(list(self.sems.allocated().values()))
        self.nc.all_engine_barrier()

    tile.TileContext._drain_and_barrier = _patched_drain_and_barrier

    def legalize_sync_waits(nc):
        for f in nc.m.functions:
            for bb in f.blocks:
                il = bb.instructions
                if not any(
                    inst.sync_info is not None
                    and len(inst.sync_info.on_wait or []) > 1
                    for inst in il
                ):
                    continue
                new = []
                for inst in il:
                    si = inst.sync_info
                    waits = list(si.on_wait or []) if si is not None else []
                    if len(waits) > 1 and inst.engine != mybir.EngineType.Unassigned:
                        eng = nc.engines[inst.engine]
                        for w in waits[:-1]:
                            nop = eng.nop()
                            nopmi = nop.ins
                            cur = nc.cur_bb.bb if hasattr(nc.cur_bb, 'bb') else nc.cur_bb
                            cil = cur.instructions
                            for k in range(len(cil) - 1, -1, -1):
                                if cil[k].name == nopmi.name:
                                    del cil[k]
                                    break
                            si2 = nopmi.sync_info
                            if si2 is None:
                                nopmi.sync_info = bass_rust.SyncInfo(
                                    on_wait=[w], on_update=[])
                            else:
                                si2.on_wait = [w]
                            new.append(nopmi)
                        si.on_wait = waits[-1:]
                    new.append(inst)
                il[:] = new

    F32 = mybir.dt.float32
    F32R = mybir.dt.float32r
    nc = bass.Bass('TRN2', target_bir_lowering=False, debug=False)

    xqt = nc.dram_tensor('xqt', [D, S], F32, kind='ExternalInput').ap()
    xkt = nc.dram_tensor('xkt', [D, S], F32, kind='ExternalInput').ap()
    xvt = nc.dram_tensor('xvt', [D, S], F32, kind='ExternalInput').ap()
    wq = nc.dram_tensor('wq', [D, DH], F32, kind='ExternalInput').ap()
    wk = nc.dram_tensor('wk', [D, DH], F32, kind='ExternalInput').ap()
    wv = nc.dram_tensor('wv', [D, DH], F32, kind='ExternalInput').ap()
    bq = nc.dram_tensor('bq', [128, 4], F32, kind='ExternalInput').ap()
    bk = nc.dram_tensor('bk', [128, 4], F32, kind='ExternalInput').ap()
    wo = nc.dram_tensor('wo', [DH, D], F32, kind='ExternalInput').ap()
    out = nc.dram_tensor('out', [S, 4 * D], F32, kind='ExternalOutput').ap()

    EXP = mybir.ActivationFunctionType.Exp

    with tile.TileContext(nc) as tc:
        with tc.tile_pool(name='const', bufs=1) as constp, \
             tc.tile_pool(name='qkv', bufs=1) as qkv, \
             tc.tile_pool(name='atp', bufs=3) as atp, \
             tc.tile_pool(name='bst', bufs=3) as bst, \
             tc.tile_pool(name='drp', bufs=1, space='DRAM') as drp, \
             tc.tile_pool(name='pa', bufs=2, space='PSUM') as pa, \
             tc.tile_pool(name='pb', bufs=4, space='PSUM') as pb:

            ones_f32 = constp.tile([128, 128], F32, name='ones_f32')
            nc.vector.memset(ones_f32[:], 1.0)
            bq_t = constp.tile([128, 4], F32, name='bq_t')
            bk_t = constp.tile([128, 4], F32, name='bk_t')
            nc.sync.dma_start(bq_t[:], bq[:])
            nc.sync.dma_start(bk_t[:], bk[:])
            # head h's denominators live at partition (h//2)*32 + h%2
            # (DVE partition offsets must be 32-aligned)
            sums = constp.tile([128, S], F32, name='sums')
            scratch = drp.tile([HPC, S], F32, name='scratch')

            # persistent activation tiles
            qT = [qkv.tile([128, S], F32R, name=f'qT{j}', tag=f'qT{j}')
                  for j in range(4)]
            kT = [qkv.tile([128, S], F32R, name=f'kT{j}', tag=f'kT{j}')
                  for j in range(4)]
            OT = [qkv.tile([128, S], F32R, name=f'OT{j}', tag=f'OT{j}')
                  for j in range(4)]
            v_aug = qkv.tile([128, HPC * NKB * 65], F32R, name='v_aug',
                             tag='v_aug')
            v_view = v_aug.rearrange('p (h c w) -> p h c w', h=HPC, c=NKB)
            nc.vector.tensor_copy(
                v_view[:, :, :, 64:65],
                ones_f32.rearrange('p (h c w) -> p h c w', h=HPC, c=NKB))

            # ---------------- projections ----------------
            w_dram = {0: wq, 1: wk, 2: wv}
            with tc.tile_pool(name='wch', bufs=8) as wch, \
                 tc.tile_pool(name='xtp', bufs=2) as xtp:
                for pi in range(3):
                    xsrc = {0: xqt, 1: xkt, 2: xvt}[pi]
                    xsrc_v = xsrc.bitcast(F32R).rearrange(
                        '(c p) s -> p c s', p=128)
                    wt = []
                    for sc in range(NSC):
                        xt = xtp.tile([128, 8 * SC], F32R, name='xt', tag='xt')
                        xtv = xt.rearrange('p (c n) -> p c n', c=8)
                        for kc in range(8):
                            if sc == 0:
                                w1 = wch.tile([128, DH], F32R,
                                              name=f'w{pi}_{kc}', tag='w')
                                nc.sync.dma_start(
                                    w1[:],
                                    w_dram[pi].bitcast(F32R)
                                    [kc * 128:(kc + 1) * 128, :])
                                wt.append(w1)
                            nc.sync.dma_start(
                                xtv[:, kc, :],
                                xsrc_v[:, kc, sc * SC:(sc + 1) * SC])
                        if pi < 2:
                            dst = qT if pi == 0 else kT
                            bias = bq_t if pi == 0 else bk_t
                            for j in range(4):
                                acc = pb.tile([128, 512], F32, name='acc',
                                              tag='pb')
                                for kc in range(8):
                                    nc.tensor.matmul(
                                        acc[:],
                                        wt[kc][:, j * 128:(j + 1) * 128],
                                        xtv[:, kc, :],
                                        start=(kc == 0), stop=(kc == 7))
                                nc.vector.tensor_scalar_add(
                                    dst[j][:, sc * SC:(sc + 1) * SC], acc[:],
                                    bias[:, j:j + 1])
                        else:
                            for rb in range(4):
                                acc = pb.tile([128, 512], F32, name='acc',
                                              tag='pb')
                                for kc in range(8):
                                    nc.tensor.matmul(
                                        acc[:],
                                        xtv[:, kc, rb * 128:(rb + 1) * 128],
                                        wt[kc][:],
                                        start=(kc == 0), stop=(kc == 7))
                                cg = sc * 4 + rb
                                nc.vector.tensor_copy(
                                    v_view[:, :, cg, 0:64],
                                    acc.rearrange('p (h d) -> p h d', h=HPC))

            # ------------- attention + pipelined outproj -------------
            with tc.tile_pool(name='scp', bufs=3) as scp, \
                 tc.tile_pool(name='oevp', bufs=3) as oevp, \
                 tc.tile_pool(name='wop', bufs=1) as wop:
                wo_t = wop.tile([128, 4 * D], F32R, name='wo_t', tag='wo')
                nc.sync.dma_start(
                    wo_t.rearrange('p (c n) -> p c n', c=4),
                    wo.bitcast(F32R).rearrange('(c p) n -> p c n', p=128))

                oev_state = {}

                def emit_op_item(item):
                    # one output-projection matmul (half a query block)
                    jp, qb, nco = item
                    if nco == 0:
                        oev_state[jp] = oevp.tile([128, D], F32, name='oev',
                                                  tag='oev')
                    oev = oev_state[jp]
                    acc = pb.tile([128, 512], F32, name='acc_o', tag='pb')
                    nc.tensor.matmul(
                        acc[:],
                        OT[jp][:, qb * 128:(qb + 1) * 128],
                        wo_t[:, jp * D + nco * 512:jp * D + (nco + 1) * 512],
                        start=True, stop=True)
                    nc.vector.tensor_copy(
                        oev[:, nco * 512:(nco + 1) * 512], acc[:])
                    if nco == 1:
                        nc.sync.dma_start(
                            out[qb * 128:(qb + 1) * 128,
                                jp * D:(jp + 1) * D], oev[:])

                def outproj(jp, qbs=range(16)):
                    for qb in qbs:
                        for nco in range(2):
                            emit_op_item((jp, qb, nco))

                for j in range(4):
                    for hi in range(2):
                        h = 2 * j + hi
                        po = hi * 64
                        for qcg in range(2):
                            accs = []
                            for qh in range(2):
                                a = pb.tile([128, 512], F32, name='acc_b',
                                            tag='pb')
                                accs.append(a)
                            for kb in range(NKB):
                                if j > 0 and hi == 0 and qcg == 1:
                                    if kb == 5:
                                        outproj(j - 1, range(8))
                                    elif kb == 11:
                                        outproj(j - 1, range(8, 16))
                                if j == 3 and hi == 1 and qcg == 1:
                                    if kb == 5:
                                        outproj(3, range(4))
                                    elif kb == 11:
                                        outproj(3, range(4, 8))
                                pw = pa.tile([128, 1024], F32, name='pw',
                                             tag='pa')
                                for qh in range(2):
                                    nc.tensor.matmul(
                                        pw[:, qh * 512:(qh + 1) * 512],
                                        kT[j][po:po + 64,
                                              kb * 128:(kb + 1) * 128],
                                        qT[j][po:po + 64,
                                              qcg * 1024 + qh * 512:
                                              qcg * 1024 + (qh + 1) * 512],
                                        start=True, stop=True)
                                at = atp.tile([128, 1024], F32R, name='at',
                                              tag='at')
                                nc.scalar.activation(at[:], pw[:], EXP)
                                for qh in range(2):
                                    nc.tensor.matmul(
                                        accs[qh][0:65, :],
                                        v_aug[:, (h * NKB + kb) * 65:
                                              (h * NKB + kb) * 65 + 65],
                                        at[:, qh * 512:(qh + 1) * 512],
                                        start=(kb == 0), stop=(kb == NKB - 1))
                            for qh in range(2):
                                qc = qcg * 2 + qh
                                st = bst.tile([65, 512], F32R, name='st',
                                              tag='bst')
                                nc.vector.tensor_copy(st[0:65, :],
                                                      accs[qh][0:65, :])
                                nc.sync.dma_start(
                                    OT[j][po:po + 64,
                                          qc * 512:(qc + 1) * 512],
                                    st[0:64, :])
                                srow = j * 32 + hi
                                nc.sync.dma_start(
                                    sums[srow:srow + 1,
                                         qc * 512:(qc + 1) * 512],
                                    st[64:65, :].bitcast(F32))
                                if hi == 1:
                                    # both heads done for this qc: normalize
                                    nc.vector.reciprocal(
                                        sums[32 * j:32 * j + 2,
                                             qc * 512:(qc + 1) * 512],
                                        sums[32 * j:32 * j + 2,
                                             qc * 512:(qc + 1) * 512])
                                    nc.sync.dma_start(
                                        scratch[2 * j:2 * j + 2,
                                                qc * 512:(qc + 1) * 512],
                                        sums[32 * j:32 * j + 2,
                                             qc * 512:(qc + 1) * 512])
                                    sc_t = scp.tile([128, 512], F32,
                                                    name='sc_t', tag='sc')
                                    nc.sync.dma_start(
                                        sc_t[0:64, :],
                                        scratch[2 * j:2 * j + 1,
                                                qc * 512:(qc + 1) * 512]
                                        .partition_broadcast(64))
                                    nc.sync.dma_start(
                                        sc_t[64:128, :],
                                        scratch[2 * j + 1:2 * j + 2,
                                                qc * 512:(qc + 1) * 512]
                                        .partition_broadcast(64))
                                    nc.vector.tensor_mul(
                                        OT[j][:, qc * 512:(qc + 1) * 512],
                                        OT[j][:, qc * 512:(qc + 1) * 512],
                                        sc_t[:])
                                    if j == 3 and qcg == 1:
                                        # tail: emit outproj for the newly
                                        # normalized query range right away
                                        outproj(3, range(8 + 4 * qh,
                                                         12 + 4 * qh))

    legalize_sync_waits(nc)
    return nc


def _get_nc():
    if 'nc' not in _CACHE:
        _CACHE['nc'] = _build()
    return _CACHE['nc']


def _make_in_maps(Q, K, V, Wq, bq, Wk, bk, Wv, bv, Wo):
    f32 = np.float32
    Q = np.asarray(Q, f32)
    K = np.asarray(K, f32)
    V = np.asarray(V, f32)
    Wq = np.asarray(Wq, f32)
    Wk = np.asarray(Wk, f32)
    Wv = np.asarray(Wv, f32)
    Wo = np.asarray(Wo, f32)
    bq = np.asarray(bq, f32)
    bk = np.asarray(bk, f32)
    bv = np.asarray(bv, f32)
    scale = f32(1.0 / np.sqrt(DK))
    in_maps = []
    for c in range(N_CORES):
        b, g = c // 2, c % 2
        cs = slice(g * DH, (g + 1) * DH)
        in_maps.append({
            'xqt': np.ascontiguousarray(Q[b].T),
            'xkt': np.ascontiguousarray(K[b].T),
            'xvt': np.ascontiguousarray(V[b].T),
            'wq': np.ascontiguousarray(Wq[:, cs] * scale),
            'wk': np.ascontiguousarray(Wk[:, cs]),
            'wv': np.ascontiguousarray(Wv[:, cs]),
            'bq': np.ascontiguousarray((bq[cs] * scale).reshape(4, 128).T),
            'bk': np.ascontiguousarray(bk[cs].reshape(4, 128).T),
            'wo': np.ascontiguousarray(Wo[cs, :]),
        })
    return in_maps


def _run(in_maps, trace=False, tmpdir=None):
    from concourse import bass_utils
    nc = _get_nc()
    kw = {}
    if trace:
        kw = dict(trace=True, tmpdir=tmpdir)
    return bass_utils.run_bass_kernel_spmd(
        nc, in_maps, core_ids=list(range(N_CORES)), **kw)


def kernel(Q, K, V, Wq, bq, Wk, bk, Wv, bv, Wo, bo):
    in_maps = _make_in_maps(Q, K, V, Wq, bq, Wk, bk, Wv, bv, Wo)
    res = _run(in_maps)
    # V-bias passes through softmax (attention rows sum to 1), so its
    # contribution is the constant row bv @ Wo, added here exactly.
    const_row = (np.asarray(bv, np.float64) @ np.asarray(Wo, np.float64)
                 + np.asarray(bo, np.float64)).astype(np.float32)
    outs = [r['out'].reshape(S, 4, D).sum(axis=1) for r in res.results]
    full = np.stack(
        [outs[2 * b] + outs[2 * b + 1] + const_row[None, :]
         for b in range(B)], axis=0)
    return full.astype(np.float32)

